# revision 12
# baseline (speedup 1.0000x reference)
"""Causal single-head self-attention on 8 TRN2 NeuronCores.

Sharding: 8 cores = 4 batches x 2 cores/batch. Within a batch the 8
512-query chunks are split zigzag (core A owns chunks {0,3,4,7}, core B
{1,2,5,6}) so causal work balances. Each core projects K/V for the
whole batch from its own copy of x (recompute beats cross-core K/V
exchange: a pair AllGather measured ~8us per collective plus ~29us
cc-engine warmup), computes Q only for its owned chunks, then does
block-causal flash-style attention without the row-max pass (scores
are O(1) so exp never overflows) and a fused out-projection.

SPMD trick: one program runs on all 8 cores; per-core differences live
in the DATA only. x rows are fed in a per-core storage permutation that
puts owned query chunks at storage chunks 0,2,4,6. The diagonal
k-blocks use 4 SHARED staircase masks (gpsimd affine_select); partner
k-blocks are all-keep or all-drop via a per-core per-slot bias column
on the Exp activation (exp(s*scale - 50) == 0 in bf16).

v2 schedule (vs the first working version at 129.3us):
 - x is passed chunk-contiguous ([p, kt, c, s] bf16) so each chunk-half
   load is one 4KB-per-partition descriptor chain; the strided D-major
   layout only reached ~141GB/s and starved the PE at startup.
 - 8 warm-up matmuls on a memset tile run while the first x chunk
   lands, so the HAM clock-gate reaches 2.4GHz before real work (the
   old kernel ran its first 20us at 1.2GHz).
 - EAGER attention: slot g's k-blocks run as soon as their K/V chunk is
   projected (chunks 0..2g at kt=2g, chunk 2g+1 at kt=2g+1), instead of
   the whole slot after kt=2g+1. This splits the 32-block final slot so
   only 4 blocks + the finish chain remain after the last projection
   (the old tail exposed ~15us).
 - scores->exp->PV is software-pipelined with skew 2 (PV of block n
   emitted after scores of n+2) and the V-projection/V-transpose PE ops
   are woven between attention blocks, so the PE never sits on the
   ~600ns scalar Exp; the last 2 PV groups of a slot are flushed after
   the NEXT kt's K-projection for the same reason.
 - softmax normalization rides the out-projection PSUM->SBUF copies as
   a per-partition 1/rowsum scale (halves split scalar/vector), instead
   of a separate normalize+transpose chain on the critical path.

Layouts (partition dim first):
  xT   [128, 8, 8, 512] bf16  x^T per (chunk, d-chunk)
  K^T  [128, 4096]      bf16  H-major keys
  Q^T  [128, 2048]      bf16  H-major owned queries
  V    [128, 32, 256]   bf16  token-major V tiles (PE-transposed); col
                              128 = ones column (rowsum trick)
  scores_T [k=128, q=512] PSUM; P_T = exp(scale*s + bias) bf16
  O [q=128, 128+1] accumulates in PSUM over k-blocks with P_T subtiles
  stationary and [V|1] moving; col 128 = softmax denominator.
"""

import ml_dtypes
import numpy as np
from collections import deque
from contextlib import ExitStack

import concourse.bass as bass
import concourse.tile as tile
from concourse import bacc, mybir
from concourse.bass_utils import run_bass_kernel_spmd
from concourse.masks import make_identity

S, B, D, H = 4096, 4, 1024, 128
P = 128
QC = 512                  # query chunk
NSLOT = 4                 # owned chunks per core
DC = D // P               # 8 d-chunks
TT = S // P               # 32 token tiles / k-blocks
NKT = S // QC             # 8 key 512-chunks
SCALE = float(H) ** -0.5
MASK_BIAS = -50.0         # exp(s*SCALE + MASK_BIAS) rounds to 0 in bf16
SKEW = 2                  # scores->PV software pipeline depth

# storage-order permutation of the 8 query chunks, per role. Queries the
# core owns sit at storage chunks 0,2,4,6; the first 2(g+1) storage
# chunks cover every true key needed by owned chunk g (extras masked).
SIGMA = {0: [0, 1, 3, 2, 4, 5, 7, 6], 1: [1, 0, 2, 3, 5, 4, 6, 7]}
QSLOT = [0, 2, 4, 6]      # storage chunk positions of owned queries

F32 = mybir.dt.float32
BF16 = mybir.dt.bfloat16


def _build_kernel():
    nc = bacc.Bacc("TRN2", target_bir_lowering=False, debug=False, num_devices=8)

    xb2 = nc.dram_tensor("xb2", [P, NKT, DC, QC], BF16, kind="ExternalInput")
    wqT = nc.dram_tensor("wqT", [P, DC, H], BF16, kind="ExternalInput")
    wkT = nc.dram_tensor("wkT", [P, DC, H], BF16, kind="ExternalInput")
    wvT = nc.dram_tensor("wvT", [P, DC, H], BF16, kind="ExternalInput")
    woT = nc.dram_tensor("woT", [H, D], BF16, kind="ExternalInput")
    meta = nc.dram_tensor("meta", [P, NSLOT], F32, kind="ExternalInput")
    out = nc.dram_tensor("out", [NSLOT * QC, D], BF16, kind="ExternalOutput")

    with ExitStack() as ctx:
        tc = ctx.enter_context(tile.TileContext(nc))
        _body(ctx, tc, xb2.ap(), wqT.ap(), wkT.ap(), wvT.ap(), woT.ap(),
              meta.ap(), out.ap())

    nc.compile()
    return nc


def _body(ctx, tc, xb2, wqT, wkT, wvT, woT, meta, out):
    nc = tc.nc

    consts = ctx.enter_context(tc.tile_pool(name="consts", bufs=1))
    bigbuf = ctx.enter_context(tc.tile_pool(name="bigbuf", bufs=1))
    ptpool = ctx.enter_context(tc.tile_pool(name="pt", bufs=8))
    otmp_pool = ctx.enter_context(tc.tile_pool(name="otmp", bufs=4))
    ypool = ctx.enter_context(tc.tile_pool(name="y", bufs=6))
    psA = ctx.enter_context(tc.tile_pool(name="psA", bufs=4, space="PSUM"))
    psTr = ctx.enter_context(tc.tile_pool(name="psTr", bufs=2, space="PSUM"))
    psO = ctx.enter_context(tc.tile_pool(name="psO", bufs=2, space="PSUM"))

    xT = bigbuf.tile([P, NKT, DC, QC], BF16)
    k_sb = bigbuf.tile([P, S], BF16)
    vT_sb = bigbuf.tile([P, S], BF16)
    q_sb = bigbuf.tile([P, NSLOT * QC], BF16)
    v_sb = bigbuf.tile([P, TT, 2 * P], BF16)  # V k-blocks + ones col
    o_t = bigbuf.tile([P, NSLOT * NSLOT, P], BF16)  # O^T [h, q-tile, q]
    rec_sb = bigbuf.tile([P, NSLOT * NSLOT], F32)   # 1/rowsum per q-tile

    wk_sb = consts.tile([P, DC, H], BF16)
    wv_sb = consts.tile([P, DC, H], BF16)
    wq_sb = consts.tile([P, DC, H], BF16)
    woT_sb = consts.tile([P, D], BF16)
    meta_sb = consts.tile([P, NSLOT], F32)
    warm = consts.tile([P, QC], BF16)

    # PE warm-up: keep the HAM activity monitor busy while the first x
    # chunk streams in, so real projections start at 2.4GHz.
    nc.gpsimd.memset(warm[:], 0.015625)
    ps_w = psA.tile([P, QC], F32, name="ps")
    for i in range(4):
        nc.tensor.matmul(ps_w[:], lhsT=warm[:, 0:P], rhs=warm[:],
                         start=(i == 0), stop=(i == 3))

    # DMA: weights + all x chunks upfront, chunk-halves split across the
    # two hardware queues (sync gets c4:8 + weights, scalar gets c0:4).
    nc.sync.dma_start(wk_sb[:], wkT)
    nc.scalar.dma_start(xT[:, 0, 0:4, :], xb2[:, 0, 0:4, :])
    nc.sync.dma_start(xT[:, 0, 4:8, :], xb2[:, 0, 4:8, :])
    nc.scalar.dma_start(xT[:, 1, 0:4, :], xb2[:, 1, 0:4, :])
    nc.sync.dma_start(xT[:, 1, 4:8, :], xb2[:, 1, 4:8, :])
    nc.scalar.dma_start(meta_sb[:], meta)
    nc.sync.dma_start(wv_sb[:], wvT)
    nc.scalar.dma_start(xT[:, 2, 0:4, :], xb2[:, 2, 0:4, :])
    nc.sync.dma_start(xT[:, 2, 4:8, :], xb2[:, 2, 4:8, :])
    nc.sync.dma_start(wq_sb[:], wqT)
    nc.sync.dma_start(woT_sb[:], woT)
    for kt in range(3, NKT):
        eng = nc.scalar if kt % 2 == 0 else nc.sync
        eng.dma_start(xT[:, kt, :, :], xb2[:, kt, :, :])

    # 4 shared diagonal staircase masks: mask_j[k, q'] = 1 iff q' >= k + 128j
    mask_sb = consts.tile([P, 4, QC], BF16)
    for j in range(4):
        nc.gpsimd.memset(mask_sb[:, j, :], 1.0)
        nc.gpsimd.affine_select(
            out=mask_sb[:, j, :], in_=mask_sb[:, j, :],
            compare_op=mybir.AluOpType.is_ge, fill=0.0,
            base=-128 * j, pattern=[[1, QC]], channel_multiplier=-1)
    ident = consts.tile([P, P], BF16)
    make_identity(nc, ident[:])
    nc.vector.memset(v_sb[:, :, H], 1.0)  # ones column for rowsum trick

    def proj_mms(w_sb, kt):
        ps = psA.tile([P, QC], F32, name="ps")
        for c in range(DC):
            nc.tensor.matmul(ps[:], lhsT=w_sb[:, c, :],
                             rhs=xT[:, kt, c, :],
                             start=(c == 0), stop=(c == DC - 1))
        return ps

    def proj(w_sb, dst, kt, dst_kt=None):
        ps = proj_mms(w_sb, kt)
        nc.vector.tensor_copy(
            dst[:, bass.ts(kt if dst_kt is None else dst_kt, QC)], ps[:])

    def vtrans(bk):
        pstr = psTr.tile([P, P], BF16, name="tr")
        nc.tensor.transpose(pstr[:], vT_sb[:, bass.ts(bk, P)], ident[:])
        nc.vector.tensor_copy(v_sb[:, bk, 0:H], pstr[:])

    # ---- attention pipeline state ----
    po_pk = [None, None]  # current slot's O PSUM tiles (2 subs per bank)
    pend = deque()        # (bk, pt, po_tiles) awaiting PV emission
    nb_cur = [0]          # block count of current slot

    def emit_scores(g, c, j):
        bk = 4 * c + j
        # diagonal block j is fully masked for q-subtiles < j: trim the
        # scores/exp/mask free extent to q >= 128j and skip those subs' PV
        lo = 128 * j if c == 2 * g else 0
        ps = psA.tile([P, QC], F32, name="ps")
        nc.tensor.matmul(ps[:, lo:QC], lhsT=k_sb[:, bass.ts(bk, P)],
                         rhs=q_sb[:, g * QC + lo : (g + 1) * QC],
                         start=True, stop=True)
        pt = ptpool.tile([P, QC], BF16)
        if c == 2 * g + 1:   # partner chunk: all-keep/all-drop via bias
            nc.scalar.activation(pt[:], ps[:],
                                 mybir.ActivationFunctionType.Exp,
                                 bias=meta_sb[:, g : g + 1], scale=SCALE)
        else:
            nc.scalar.activation(pt[:, lo:QC], ps[:, lo:QC],
                                 mybir.ActivationFunctionType.Exp,
                                 scale=SCALE)
        if c == 2 * g:   # staircase only spans the leading q-subtile
            nc.vector.tensor_mul(pt[:, lo : lo + P], pt[:, lo : lo + P],
                                 mask_sb[:, j, lo : lo + P])
        pend.append((bk, pt, tuple(po_pk), lo // P))

    def emit_pv():
        bk, pt, po_t, sub0 = pend.popleft()
        for sub in range(sub0, NSLOT):
            # start/stop clear/close the whole 2KB PSUM zero-region, so
            # only the first/last matmul touching each bank carries them;
            # the second accumulator in the bank relies on per-element
            # pending-zero after the bank clear.
            nc.tensor.matmul(po_t[sub // 2][:, sub % 2, :],
                             lhsT=pt[:, bass.ts(sub, P)],
                             rhs=v_sb[:, bk, 0 : H + 1],
                             start=(bk == 0 and sub % 2 == 0),
                             stop=(bk == nb_cur[0] - 1 and sub % 2 == 1))

    def fin_stages(g, sub, po_t, tail=False):
        """Finish chain for one q-subtile, split into stages so the PE
        parts can be woven between attention blocks (the PE queue is
        in-order; a monolithic chain stalls it on vector copies)."""
        idx = g * NSLOT + sub
        pos = po_t[sub // 2][:, sub % 2, :]
        st = {}

        def a():  # vector: reciprocal + raw O copy (no normalize)
            nc.vector.reciprocal(rec_sb[:, idx : idx + 1], pos[:, H : H + 1])
            ob = otmp_pool.tile([P, P], BF16, name="ob")
            nc.vector.tensor_copy(ob[:], pos[:, 0:H])
            st["ob"] = ob

        def b():  # PE transpose
            pstr = psTr.tile([P, P], BF16, name="tr")
            nc.tensor.transpose(pstr[:], st["ob"][:], ident[:])
            st["tr"] = pstr

        def c():  # vector: O^T to SBUF
            nc.vector.tensor_copy(o_t[:, idx, :], st["tr"][:])

        def d():  # out-proj half 0; normalization rides the copy
            y = ypool.tile([P, D], BF16)
            st["y"] = y
            ps0 = psA.tile([P, QC], F32, name="ps")
            nc.tensor.matmul(ps0[:], lhsT=o_t[:, idx, :],
                             rhs=woT_sb[:, 0:QC], start=True, stop=True)
            if tail:  # scalar is idle at the tail; busy with exps mid-run
                nc.scalar.activation(y[:, 0:QC], ps0[:],
                                     mybir.ActivationFunctionType.Copy,
                                     scale=rec_sb[:, idx : idx + 1])
            else:
                nc.vector.tensor_scalar_mul(y[:, 0:QC], ps0[:],
                                            rec_sb[:, idx : idx + 1])

        def e():  # out-proj half 1; one whole-y DMA on alternating rings
            y = st["y"]
            ps1 = psA.tile([P, QC], F32, name="ps")
            nc.tensor.matmul(ps1[:], lhsT=o_t[:, idx, :],
                             rhs=woT_sb[:, QC:D], start=True, stop=True)
            nc.vector.tensor_scalar_mul(y[:, QC:D], ps1[:],
                                        rec_sb[:, idx : idx + 1])
            eng = nc.sync if sub % 2 == 0 else nc.scalar
            eng.dma_start(out[bass.ts(idx, P), :], y[:])

        return a, [b, c, d, e]

    def mk_vproj_fillers(kt):
        """V projection + V transpose for chunk kt as single-op closures."""
        psv = [None]

        def mk_vmm(c):
            def f():
                if c == 0:
                    psv[0] = psA.tile([P, QC], F32, name="ps")
                nc.tensor.matmul(psv[0][:], lhsT=wv_sb[:, c, :],
                                 rhs=xT[:, kt, c, :],
                                 start=(c == 0), stop=(c == DC - 1))
                if c == DC - 1:
                    nc.vector.tensor_copy(vT_sb[:, bass.ts(kt, QC)], psv[0][:])
            return f

        return ([mk_vmm(c) for c in range(DC)]
                + [lambda j=j: vtrans(4 * kt + j) for j in range(4)])

    # ---- main emission loop over key 512-chunks ----
    for kt in range(NKT):
        g = kt // 2
        if kt % 2 == 0:
            proj(wk_sb, k_sb, kt)
            fillers = []
            if kt == 0:
                proj(wv_sb, vT_sb, 0)
                proj(wq_sb, q_sb, 0, dst_kt=0)
                for j in range(4):
                    vtrans(j)
            else:
                # flush previous slot's last PV groups behind fresh MMs
                while pend:
                    emit_pv()
                proj(wq_sb, q_sb, kt, dst_kt=g)
                # previous slot's finish: vector-only stage now, PE
                # stages woven between the attention blocks below
                po_old = tuple(po_pk)
                for sub in range(NSLOT):
                    a, rest = fin_stages(g - 1, sub, po_old)
                    a()
                    fillers.extend(rest)
                fillers = mk_vproj_fillers(kt) + fillers
            # new slot: O PSUM accumulators
            for h2 in range(2):
                po_pk[h2] = psO.tile([P, 2, H + 1], F32, name="po")
            nb_cur[0] = 8 * (g + 1)
            # a few fillers up front cover the q-copy latency before the
            # first scores matmul of the new slot
            for _ in range(3):
                if fillers:
                    fillers.pop(0)()
            # eager blocks: chunks 0..2g, fillers spread evenly so the PE
            # has independent work between PV groups all the way through
            nblk = 4 * (2 * g + 1)
            rate = len(fillers) / nblk if nblk else 0.0
            acc = 0.0
            for c in range(2 * g + 1):
                for j in range(4):
                    emit_scores(g, c, j)
                    acc += rate
                    while acc >= 1.0 and fillers:
                        fillers.pop(0)()
                        acc -= 1.0
                    if len(pend) > SKEW:
                        emit_pv()
            for f in fillers:
                f()
        else:
            proj(wk_sb, k_sb, kt)
            proj(wv_sb, vT_sb, kt)
            for j in range(4):
                vtrans(4 * kt + j)
            for j in range(4):
                emit_scores(g, kt, j)
                if len(pend) > SKEW:
                    emit_pv()
    while pend:
        emit_pv()
    # tail finish, stage-major so the engines pipeline across subs
    po_old = tuple(po_pk)
    stages = []
    for sub in range(NSLOT):
        a, rest = fin_stages(NSLOT - 1, sub, po_old, tail=True)
        a()
        stages.append(rest)
    for sub in range(NSLOT):
        stages[sub][0]()          # transpose
        stages[sub][1]()          # O^T copy
    for sub in range(NSLOT):
        stages[sub][2]()          # out-proj half 0
    for sub in range(NSLOT):
        stages[sub][3]()          # out-proj half 1


_CACHED_NC = None


def _get_nc():
    global _CACHED_NC
    if _CACHED_NC is None:
        _CACHED_NC = _build_kernel()
    return _CACHED_NC


def _make_core_inputs(x, wqT, wkT, wvT, woT, core):
    b, role = core // 2, core % 2
    sigma = SIGMA[role]
    perm = np.concatenate([np.arange(QC) + c * QC for c in sigma])
    xp = np.asarray(x, np.float32)[perm, b, :]          # (S, D)
    # (kt, s, c, p) -> (p, kt, c, s): chunk-contiguous for fat DMA lines
    xb = xp.reshape(NKT, QC, DC, P).transpose(3, 0, 2, 1)
    xb2 = np.ascontiguousarray(xb.astype(ml_dtypes.bfloat16))

    # per-slot partner-block bias: partner true chunk before own -> keep
    # (0), after own -> fully masked (exp(s*SCALE - 50) == 0 in bf16)
    meta = np.zeros((P, NSLOT), np.float32)
    for g in range(NSLOT):
        if sigma[2 * g + 1] > sigma[2 * g]:
            meta[:, g] = MASK_BIAS
    return {"xb2": xb2, "wqT": wqT, "wkT": wkT, "wvT": wvT, "woT": woT,
            "meta": meta}


def _w_pch(w):
    """(H, D) weight -> [p, c, h] bf16 layout for a contiguous SBUF load."""
    return np.ascontiguousarray(
        np.asarray(w, np.float32).T.reshape(DC, P, H).transpose(1, 0, 2)
        .astype(ml_dtypes.bfloat16))


def kernel(x, Wq, Wk, Wv, Wo):
    x = np.asarray(x, dtype=np.float32)
    wqT = _w_pch(Wq)
    wkT = _w_pch(Wk)
    wvT = _w_pch(Wv)
    woT = np.ascontiguousarray(np.asarray(Wo, np.float32).T
                               .astype(ml_dtypes.bfloat16))

    nc = _get_nc()
    in_maps = [_make_core_inputs(x, wqT, wkT, wvT, woT, i) for i in range(8)]
    res = run_bass_kernel_spmd(nc, in_maps, list(range(8))).results

    out = np.empty((S, B, D), np.float32)
    for core in range(8):
        b, role = core // 2, core % 2
        sigma = SIGMA[role]
        co = np.asarray(res[core]["out"], dtype=np.float32)
        for g in range(NSLOT):
            c_g = sigma[QSLOT[g]]
            out[c_g * QC : (c_g + 1) * QC, b, :] = co[g * QC : (g + 1) * QC, :]
    return out
